# revision 1
# baseline (speedup 1.0000x reference)
"""Trainium2 Bass kernel for nn_PredictionModel (CPC-style prediction scores).

Computation (B=4, L=512, D=512, C=256, K=12, LW=500):
  c_proj[b,l,k,d] = sum_c Wk[k,d,c] * c[b,l,c]          (l < LW)
  zw[b,l,k,d]     = z[b, l+1+k, d]
  pos[b,l,k]      = <c_proj[b,l,k], zw[b,l,k]>
  neg_g[b,n,l,k]  = <c_proj[b,l,k], zw[perm_B[n], perm_L[l], k]>
  neg_len[b,n,l,k]= <c_proj[b,l,k], zw[b, perms_len[n,l], k]>
  out = concat([pos[:,None], neg_g, neg_len], axis=1)   # (B, 9, LW, K)

Sharding: 8 cores = 4 batches x 2 l-ranges ([0,256) and [244,500), padded to
256 rows each; host takes l<250 from half 0 and l>=250 from half 1).
Uniform program; all per-core differences arrive via input tensors.
"""

import numpy as np
import ml_dtypes

import concourse.mybir as mybir
from concourse import bacc
from concourse.tile import TileContext
from concourse import bass_utils

B, L, D, C, K = 4, 512, 512, 256, 12
LW = L - K          # 500
LH = 256            # padded per-core l count
L0S = [0, 244]      # absolute start of each half
NM = 2 * B + 1      # 9 score rows per (l, k)
F32 = mybir.dt.float32
BF16 = mybir.dt.bfloat16
BF16_NP = ml_dtypes.bfloat16


_NC = None

# tuning knobs (sim-ablation support)
CFG = {
    "mul_dve_every": 1,   # u % N == N-1 -> DVE mul, else gpsimd (1 = all DVE)
    "red_act_every": 2,   # m % N == N-1 -> ACT reduce, else DVE
    "kg": 6,              # k's per c_proj group
    "do_mul": True,
    "do_reduce": True,
    "do_dots_dma": True,
    "halving_add": True,
    "gp_units": (),
    "halving_act": True,
    "halving2": True,
    "act_ms": None,
}


def _build_program(cfg=None):
    """One NeuronCore program, identical across the 8 cores."""
    global _NC
    if cfg is not None:
        pass
    elif _NC is not None:
        return _NC
    cfg = {**CFG, **(cfg or {})}
    nc = bacc.Bacc()
    # [c-part 128, c-chunk 2, l 256] stationary operand (c[b,half].T)
    ct_d = nc.dram_tensor("ct", [128, 2, LH], F32, kind="ExternalInput")
    # [c-part 128, k 12, c-chunk 2, d 512] moving operand (Wk[k].T)
    wkt_d = nc.dram_tensor("wkt", [128, K, 2, D], F32, kind="ExternalInput")
    # pre-gathered z windows, bf16: [m 9, blk 2, l-part 128, k 12, d 512]
    zw_d = nc.dram_tensor("zw", [NM, 2, 128, K, D], BF16, kind="ExternalInput")
    out_d = nc.dram_tensor("out", [2, 2, 128, NM * K], F32, kind="ExternalOutput")

    with TileContext(nc) as tc:
        with (
            tc.tile_pool(name="const", bufs=1) as const_pool,
            tc.tile_pool(name="cproj", bufs=1) as cproj_pool,
            tc.tile_pool(name="psum", bufs=8, space="PSUM") as psum_pool,
            tc.tile_pool(name="zw", bufs=cfg.get("zw_bufs", 6)) as zw_pool,
            tc.tile_pool(name="prod", bufs=5) as prod_pool,
            tc.tile_pool(name="half", bufs=6) as half_pool,
            tc.tile_pool(name="junk", bufs=4) as junk_pool,
            tc.tile_pool(name="scores", bufs=1) as scores_pool,
        ):
            ct_sb = const_pool.tile([128, 2, LH], F32, tag="ct", name="ct_sb")
            nc.sync.dma_start(out=ct_sb[:], in_=ct_d[:])
            wkt_sb = const_pool.tile([128, K, 2, D], F32, tag="wkt", name="wkt_sb")
            for kg_ in range(3):
                nc.sync.dma_start(
                    out=wkt_sb[:, kg_ * 4 : (kg_ + 1) * 4],
                    in_=wkt_d[:, kg_ * 4 : (kg_ + 1) * 4],
                )

            # c_proj[(blk, kg)]: [l 128, KG k's, d 512] fp32 matmuls -> bf16
            KG = cfg.get("kg", 4)  # k's per group
            NG = K // KG
            cproj = {}
            for kg in range(NG):
                for blk in range(2):
                    cproj[(blk, kg)] = cproj_pool.tile(
                        [128, KG, D], BF16, tag=f"cp{blk}_{kg}", name=f"cp{blk}_{kg}"
                    )
            for kg in range(NG):
                for blk in range(2):
                    for ki in range(KG):
                        k = kg * KG + ki
                        ps = psum_pool.tile(
                            [128, D], F32, name=f"ps{k}_{blk}", tag="ps"
                        )
                        for ci in range(2):
                            nc.tensor.matmul(
                                ps[:],
                                ct_sb[:, ci, blk * 128 : (blk + 1) * 128],
                                wkt_sb[:, k, ci, :],
                                start=(ci == 0),
                                stop=(ci == 1),
                            )
                        # psum->sbuf cast copies on ACT (keeps DVE free)
                        nc.scalar.copy(cproj[(blk, kg)][:, ki, :], ps[:])

            scores = {}
            for par in range(2):
                for blk in range(2):
                    scores[(par, blk)] = scores_pool.tile(
                        [128, NM * K], F32, tag=f"s{par}_{blk}", name=f"s{par}_{blk}"
                    )

            # one unit = (m, blk): mul [128, K*D] + (maybe halving add) + reduce
            units = [(m, blk) for m in range(NM) for blk in range(2)]
            mde, rae = cfg["mul_dve_every"], cfg["red_act_every"]
            zts = {}
            for u, (m, blk) in enumerate(units):
                if cfg.get("merge_zw") :
                    if blk == 0:
                        ztm = zw_pool.tile(
                            [128, 2, K, D], BF16, tag="zw", name=f"ztm{m}"
                        )
                        if cfg["do_dots_dma"]:
                            dma_eng = nc.sync if m % 2 == 0 else nc.scalar
                            dma_eng.dma_start(
                                out=ztm[:],
                                in_=zw_d[m].rearrange("b p k d -> p b k d"),
                            )
                        zts[m] = ztm
                    zt = zts[m][:, blk]
                else:
                    zt = zw_pool.tile([128, K, D], BF16, tag="zw", name=f"zt{u}")
                    if cfg["do_dots_dma"]:
                        dma_eng = nc.sync if u % 2 == 0 else nc.scalar
                        dma_eng.dma_start(out=zt[:], in_=zw_d[m, blk])
                if not cfg["do_mul"]:
                    continue
                mul_eng = nc.gpsimd if u in cfg["gp_units"] else (nc.vector if (mde and u % mde == mde - 1) else nc.gpsimd)
                on_act = (m in cfg["act_ms"]) if cfg.get("act_ms") is not None else (rae and m % rae == rae - 1)
                for kg in range(NG):
                    prod = prod_pool.tile(
                        [128, KG, D], BF16, tag="prod", name=f"pr{u}_{kg}"
                    )
                    mul_eng.tensor_tensor(
                        out=prod[:],
                        in0=cproj[(blk, kg)][:],
                        in1=zt[:, kg * KG : (kg + 1) * KG, :],
                        op=mybir.AluOpType.mult,
                    )
                    if not cfg["do_reduce"]:
                        continue
                    if cfg["halving_add"] and (cfg["halving_act"] or not on_act):
                        a1 = half_pool.tile(
                            [128, KG, D // 2], BF16, tag="half", name=f"a1{u}_{kg}"
                        )
                        nc.vector.tensor_tensor(
                            out=a1[:], in0=prod[:, :, 0 : D // 2],
                            in1=prod[:, :, D // 2 : D], op=mybir.AluOpType.add,
                        )
                        red_in, rw = a1, D // 2
                        if cfg.get("halving2") and not on_act:
                            a2 = half_pool.tile(
                                [128, KG, D // 4], BF16, tag="half2",
                                name=f"a2{u}_{kg}",
                            )
                            nc.vector.tensor_tensor(
                                out=a2[:], in0=a1[:, :, 0 : D // 4],
                                in1=a1[:, :, D // 4 : D // 2],
                                op=mybir.AluOpType.add,
                            )
                            red_in, rw = a2, D // 4
                    else:
                        red_in, rw = prod, D
                    c0 = m * K + kg * KG
                    if not on_act:
                        nc.vector.tensor_reduce(
                            out=scores[(m % 2, blk)][:, c0 : c0 + KG],
                            in_=red_in[:],
                            axis=mybir.AxisListType.X,
                            op=mybir.AluOpType.add,
                        )
                    else:
                        for ki in range(KG):
                            junk = junk_pool.tile(
                                [128, rw], BF16, tag="junk", name=f"j{u}_{kg}_{ki}"
                            )
                            nc.scalar.activation(
                                out=junk[:],
                                in_=red_in[:, ki, :],
                                func=mybir.ActivationFunctionType.Copy,
                                accum_out=scores[(m % 2, blk)][
                                    :, c0 + ki : c0 + ki + 1
                                ],
                            )

            for par in range(2):
                for blk in range(2):
                    nc.sync.dma_start(
                        out=out_d[par, blk], in_=scores[(par, blk)][:]
                    )

    nc.compile()
    if cfg == CFG:
        _NC = nc
    return nc


def _make_inputs(c, z, Wk, perms_len, perm_L, perm_B):
    """Host-side sharding: per-core input dicts."""
    z_bf = z.astype(BF16_NP)
    wkt = np.ascontiguousarray(
        Wk.transpose(0, 2, 1).reshape(K, 2, 128, D).transpose(2, 0, 1, 3)
    )  # [128, K, 2, D]
    karr = np.arange(K, dtype=np.int64)[None, :]
    in_maps = []
    for b in range(B):
        for h in range(2):
            L0 = L0S[h]
            l_abs = np.arange(L0, L0 + LH, dtype=np.int64)
            ct = np.ascontiguousarray(
                c[b, L0 : L0 + LH, :].T.reshape(2, 128, LH).transpose(1, 0, 2)
            )  # [128, 2, LH]
            zw = np.empty((NM, 2, 128, K, D), dtype=BF16_NP)
            for m in range(NM):
                if m == 0:
                    sb, sl = b, l_abs
                elif m <= B:
                    sb, sl = int(perm_B[m - 1]), perm_L[l_abs].astype(np.int64)
                else:
                    sb, sl = b, perms_len[m - 1 - B, l_abs].astype(np.int64)
                rows = sl[:, None] + 1 + karr  # (LH, K)
                zw[m] = z_bf[sb, rows].reshape(2, 128, K, D)
            in_maps.append({"ct": ct, "wkt": wkt, "zw": zw})
    return in_maps


def kernel(c, z, Wk, perms_len, perm_L, perm_B, _trace=False, _result_holder=None):
    c = np.asarray(c, np.float32)
    z = np.asarray(z, np.float32)
    Wk = np.asarray(Wk, np.float32)
    perms_len = np.asarray(perms_len)
    perm_L = np.asarray(perm_L)
    perm_B = np.asarray(perm_B)

    nc = _build_program()
    in_maps = _make_inputs(c, z, Wk, perms_len, perm_L, perm_B)
    res = bass_utils.run_bass_kernel_spmd(
        nc, in_maps, core_ids=list(range(2 * B)), trace=_trace
    )
    if _result_holder is not None:
        _result_holder.append(res)

    out = np.empty((B, NM, LW, K), np.float32)
    for b in range(B):
        for h in range(2):
            r = res.results[2 * b + h]["out"].reshape(2, LH, NM, K)
            merged = np.empty((LH, NM, K), np.float32)
            for m in range(NM):
                merged[:, m] = r[m % 2, :, m]
            s = merged.transpose(1, 0, 2)
            if h == 0:
                out[b, :, :250, :] = s[:, :250, :]
            else:
                out[b, :, 250:, :] = s[:, 250 - L0S[1] :, :]
    return out



# revision 20
# speedup vs baseline: 2.1233x; 2.1233x over previous
"""Trainium2 Bass kernel for nn_PredictionModel (CPC-style prediction scores).

Computation (B=4, L=512, D=512, C=256, K=12, LW=500):
  c_proj[b,l,k,d] = sum_c Wk[k,d,c] * c[b,l,c]          (l < LW)
  zw[b,l,k,d]     = z[b, l+1+k, d]
  pos[b,l,k]      = <c_proj[b,l,k], zw[b,l,k]>
  neg_g[b,n,l,k]  = <c_proj[b,l,k], zw[perm_B[n], perm_L[l], k]>
  neg_len[b,n,l,k]= <c_proj[b,l,k], zw[b, perms_len[n,l], k]>
  out = concat([pos[:,None], neg_g, neg_len], axis=1)   # (B, 9, LW, K)

Sharding: 8 cores = 4 batches x 2 ranges ([0,256) and [244,500); host takes
position <250 from half 0 and >=250 from half 1).

All 9 score sets are computed in d-partition layout: products
P[d, k, l] = cprojT[d, k, l] * window[d, k, l] on DVE/Pool, then the
d-reduction runs on PE as indicator-column matmuls that accumulate every
(group, dchunk) product into ONE PSUM [9, K*LH] scores tile
(row = score set).

Window sources:
 * pos: in-place overlapping AP on resident zT[b] (k,l strides both 1).
 * neg_g: in-place overlapping AP on resident zT[perm_B[n]], evaluated in
   source-row order j with cprojG from host-permuted c (sigma_g =
   argsort(perm_L)); host scatters j->l afterwards (free).
 * neg_len: host-gathered d-part window tiles, DMA-streamed.

cprojT versions (identity + sigma_g) are computed by PE matmuls straight
into [d, k, l] layout chunks and cast to bf16 via ACT copies.
"""

import numpy as np
import ml_dtypes

import bass_rust
import concourse.mybir as mybir
from concourse import bacc
from concourse.tile import TileContext
from concourse import bass_utils

B, L, D, C, K = 4, 512, 512, 256, 12
LW = L - K          # 500
LH = 256            # padded per-core l (and j) count
L0S = [0, 244]      # absolute start of each half
NM = 2 * B + 1      # 9 score rows per (l, k)
F32 = mybir.dt.float32
BF16 = mybir.dt.bfloat16
BF16_NP = ml_dtypes.bfloat16

NDC = D // 128      # 4 d-part chunks
FREE = K * LH       # 3072 flattened (k, l) columns

_NC = None

# score-row order: 0=pos, 1..4=neg_g (j-order), 5..8=neg_len
# pool_mults: set of (row, dc) chunk-mult assignments run on Pool instead
# of DVE.
CFG = {
    # (row, dc, third) mults run on Pool when (row in pool_rows and
    # third in pool_thirds); everything else on DVE
    "pool_rows": (2, 6),
    "pool_thirds": (1, 2),
    "zw_bufs": 8,
    "prodg_bufs": 12,
    "copy_rot": "a",      # version psum->sbuf copy engine rotation
}


def _win(zt_sb, col, base, nk, nj):
    """Overlapping-window AP [128, nk, nj] over zt_sb[:, col, base:]:
    element (d, k, j) -> zt_sb[d, col, base + k + j]."""
    ap = zt_sb[:, col, base:].copy()
    part = ap.ap[0]
    ap.ap = bass_rust.VecI64Pair([list(part), [1, nk], [1, nj]])
    return ap


def _build_program(cfg=None):
    """One NeuronCore program, identical across the 8 cores."""
    global _NC
    if cfg is None and _NC is not None:
        return _NC
    cfg = {**CFG, **(cfg or {})}
    nc = bacc.Bacc()
    ct_d = nc.dram_tensor("ct", [128, 2, LH], BF16, kind="ExternalInput")
    ctg_d = nc.dram_tensor("ctg", [128, 2, LH], BF16, kind="ExternalInput")
    wkt_d = nc.dram_tensor("wkt", [128, K, 2, D], BF16, kind="ExternalInput")
    # z transposed: slots 0..3 = z[perm_B[n]], slot 4 = z[b] (own batch)
    zt_d = nc.dram_tensor("zt", [B + 1, NDC, 128, L], BF16, kind="ExternalInput")
    # host-gathered d-part windows for neg_len: [q 4, dc 4, d 128, k, l]
    zw_d = nc.dram_tensor("zw", [B, NDC, 128, K, LH], BF16, kind="ExternalInput")
    # all scores: row 0=pos, 1..4=neg_g, 5..8=neg_len; columns (k, l|j)
    outs_d = nc.dram_tensor("outs", [NM, FREE], F32, kind="ExternalOutput")

    pool_rows = set(cfg["pool_rows"])
    pool_thirds = set(cfg["pool_thirds"])
    NT = 3                    # k-thirds
    KT = K // NT              # 4 k's per third
    TFREE = KT * LH           # 1024 columns per third

    with TileContext(nc) as tc:
        with (
            tc.tile_pool(name="const", bufs=1) as const_pool,
            tc.tile_pool(name="psum", bufs=2, space="PSUM") as psum_pool,
            tc.tile_pool(name="psq", bufs=6, space="PSUM") as psq_pool,
            tc.tile_pool(name="cpg", bufs=1) as cpg_pool,
            tc.tile_pool(name="zw", bufs=cfg["zw_bufs"]) as zw_pool,
            tc.tile_pool(name="prodg", bufs=cfg["prodg_bufs"]) as prodg_pool,
            tc.tile_pool(name="sgc", bufs=2) as sgc_pool,
        ):
            ct_sb = const_pool.tile([128, 2, LH], BF16, tag="ct", name="ct_sb")
            nc.sync.dma_start(out=ct_sb[:], in_=ct_d[:])
            ctg_sb = const_pool.tile([128, 2, LH], BF16, tag="ctg", name="ctg_sb")
            nc.sync.dma_start(out=ctg_sb[:], in_=ctg_d[:])
            wkt_sb = const_pool.tile([128, K, 2, D], BF16, tag="wkt", name="wkt_sb")
            for kc in range(3):
                nc.sync.dma_start(
                    out=wkt_sb[:, kc * 4 : (kc + 1) * 4],
                    in_=wkt_d[:, kc * 4 : (kc + 1) * 4],
                )
            zt_sb = const_pool.tile(
                [128, (B + 1) * NDC, L], BF16, tag="zt", name="zt_sb"
            )
            nc.sync.dma_start(
                out=zt_sb[:], in_=zt_d.rearrange("s c p r -> p (s c) r")
            )
            # indicator columns: e9s[r][:, r] = 1
            e9s = []
            for r in range(NM):
                t = const_pool.tile([128, NM], BF16, tag=f"e9_{r}", name=f"e9_{r}")
                nc.vector.memset(t[:], 0)
                nc.vector.memset(t[:, r : r + 1], 1.0)
                e9s.append(t)

            # ---- cprojT versions: [d 128, k, l] bf16 per dchunk, built
            # just-in-time per k-third inside the main loop ----
            copy_engs = {
                "a": lambda o, i: nc.scalar.copy(o, i),
                "d": lambda o, i: nc.vector.tensor_copy(out=o, in_=i),
                "p": lambda o, i: nc.gpsimd.tensor_copy(out=o, in_=i),
            }
            rot = cfg["copy_rot"]
            cp_n = [0]

            cpi = [
                cpg_pool.tile([128, K, LH], BF16, tag=f"cpi{dc}", name=f"cpi{dc}")
                for dc in range(NDC)
            ]
            cpg = [
                cpg_pool.tile([128, K, LH], BF16, tag=f"cpg{dc}", name=f"cpg{dc}")
                for dc in range(NDC)
            ]

            def build_third(tiles, src_sb, dc, t):
                """Version matmuls + copy for k in [t*KT, (t+1)*KT) of chunk dc."""
                tile = tiles[dc]
                for kp in range(t * KT // 2, (t + 1) * KT // 2):
                    psv = psum_pool.tile(
                        [128, 2, LH], F32, name=f"psv{id(tiles)}_{dc}_{kp}",
                        tag="ps",
                    )
                    for k2 in range(2):
                        for ci in range(2):
                            nc.tensor.matmul(
                                psv[:, k2],
                                wkt_sb[:, kp * 2 + k2, ci,
                                       dc * 128 : (dc + 1) * 128],
                                src_sb[:, ci, :],
                                start=(ci == 0),
                                stop=(ci == 1),
                            )
                    eng = copy_engs[rot[cp_n[0] % len(rot)]]
                    eng(tile[:, kp * 2 : (kp + 1) * 2, :], psv[:])
                    cp_n[0] += 1

            # ---- neg_len window streams (dc-major = consumption order) ----
            zw_tiles = {}
            for dc in range(NDC):
                for q in range(B):
                    t = zw_pool.tile(
                        [128, K, LH], BF16, tag="zw", name=f"zw{q}_{dc}"
                    )
                    nc.sync.dma_start(out=t[:], in_=zw_d[q, dc])
                    zw_tiles[(q, dc)] = t

            # ---- products + indicator-matmul reduction, by k-thirds ----
            # a matmul's output must fit one PSUM bank (512 f32), so scores
            # accumulate into 6 k-sixth tiles [9, 512]; each third's product
            # feeds two chain matmuls.
            psqs = [
                psq_pool.tile([NM, 2 * LH], F32, name=f"psq{s}", tag="psq")
                for s in range(2 * NT)
            ]
            nmm = [[0] for _ in range(2 * NT)]
            NCHAIN = NM * NDC

            def unit(row, dc, t, cp, win_t):
                """One (row, dchunk, k-third): mult + 2 chain matmuls."""
                prodg = prodg_pool.tile(
                    [128, KT, LH], BF16, tag="prodg", name=f"pg{row}_{dc}_{t}"
                )
                on_pool = row in pool_rows and t in pool_thirds
                eng = nc.gpsimd if on_pool else nc.vector
                eng.tensor_tensor(
                    out=prodg[:],
                    in0=cp[dc][:, t * KT : (t + 1) * KT, :],
                    in1=win_t,
                    op=mybir.AluOpType.mult,
                )
                for h in range(2):
                    s = 2 * t + h
                    nc.tensor.matmul(
                        psqs[s][:],
                        e9s[row][:],
                        prodg[:, h * 2 : (h + 1) * 2, :].rearrange(
                            "p k j -> p (k j)"
                        ),
                        start=(nmm[s][0] == 0),
                        stop=(nmm[s][0] == NCHAIN - 1),
                    )
                    nmm[s][0] += 1

            # windows per (row, dc, third): in-place APs start at column
            # 1 + t*KT (k offset folds into the window base); streamed tiles
            # are sliced on k.
            for dc in range(NDC):
                for t in range(NT):
                    build_third(cpi, ct_sb, dc, t)
                    build_third(cpg, ctg_sb, dc, t)
                    unit(0, dc, t, cpi,
                         _win(zt_sb, B * NDC + dc, 1 + t * KT, KT, LH))
                    for n in range(B):
                        if not (1 + n in pool_rows and t in pool_thirds):
                            unit(1 + n, dc, t, cpg,
                                 _win(zt_sb, n * NDC + dc, 1 + t * KT, KT, LH))
                    for q in range(B):
                        if not (5 + q in pool_rows and t in pool_thirds):
                            unit(5 + q, dc, t, cpi,
                                 zw_tiles[(q, dc)][:, t * KT : (t + 1) * KT, :])
                    # pool-assigned units last within the (dc, t) phase
                    for n in range(B):
                        if 1 + n in pool_rows and t in pool_thirds:
                            unit(1 + n, dc, t, cpg,
                                 _win(zt_sb, n * NDC + dc, 1 + t * KT, KT, LH))
                    for q in range(B):
                        if 5 + q in pool_rows and t in pool_thirds:
                            unit(5 + q, dc, t, cpi,
                                 zw_tiles[(q, dc)][:, t * KT : (t + 1) * KT, :])

            # drain scores: PSUM -> SBUF chunks -> DRAM
            for s in range(2 * NT):
                sgc = sgc_pool.tile([NM, 2 * LH], F32, tag="sgc", name=f"sgc{s}")
                nc.scalar.copy(sgc[:], psqs[s][:])
                nc.sync.dma_start(
                    out=outs_d.rearrange("m (k j) -> m k j", k=K)[
                        :, s * 2 : (s + 1) * 2, :
                    ],
                    in_=sgc[:],
                )

    nc.compile()
    if cfg == CFG:
        _NC = nc
    return nc


def _make_inputs(c, z, Wk, perms_len, perm_L, perm_B):
    """Host-side sharding: per-core input dicts."""
    z_bf = z.astype(BF16_NP)
    wkt = np.ascontiguousarray(
        Wk.transpose(0, 2, 1).reshape(K, 2, 128, D).transpose(2, 0, 1, 3)
    ).astype(BF16_NP)  # [128, K, 2, D]
    # zT chunks: [NDC, 128, L] per batch
    ztc = np.ascontiguousarray(z_bf.transpose(0, 2, 1).reshape(B, NDC, 128, L))
    perm_B = np.asarray(perm_B, np.int64)
    sigma_g = np.argsort(perm_L).astype(np.int64)  # l = sigma_g[j]
    karr = np.arange(K, dtype=np.int64)
    in_maps = []
    for b in range(B):
        for h in range(2):
            L0 = L0S[h]
            l_abs = np.arange(L0, L0 + LH, dtype=np.int64)

            def ctr(cols):
                return np.ascontiguousarray(
                    c[b, cols, :].T.reshape(2, 128, LH).transpose(1, 0, 2)
                ).astype(BF16_NP)

            ct = ctr(l_abs)
            ctg = ctr(sigma_g[l_abs])
            # zt slots 0..3 = z[perm_B[n]], slot 4 = z[b]; all shifted by L0
            # so window column (1 + j + k) reads z[sb, L0 + j + 1 + k]
            zt = np.empty((B + 1, NDC, 128, L), dtype=BF16_NP)
            for n in range(B):
                sh = np.zeros((L, 512), dtype=BF16_NP)
                sh[: L - L0] = z_bf[perm_B[n], L0:]
                zt[n] = sh.T.reshape(NDC, 128, L)
            own = np.zeros((L, 512), dtype=BF16_NP)
            own[: L - L0] = z_bf[b, L0:]
            zt[B] = own.T.reshape(NDC, 128, L)
            # neg_len windows, d-part [q, dc, d, k, l]
            zw = np.empty((B, NDC, 128, K, LH), dtype=BF16_NP)
            for q in range(B):
                sl = perms_len[q, l_abs].astype(np.int64)  # (LH,)
                rows = sl[None, :] + 1 + karr[:, None]     # (K, LH)
                g = z_bf[b, rows]                          # (K, LH, 512)
                zw[q] = g.transpose(2, 0, 1).reshape(NDC, 128, K, LH)
            in_maps.append(
                {"ct": ct, "ctg": ctg, "wkt": wkt, "zw": zw, "zt": zt}
            )
    return in_maps


def kernel(c, z, Wk, perms_len, perm_L, perm_B, _trace=False, _result_holder=None):
    c = np.asarray(c, np.float32)
    z = np.asarray(z, np.float32)
    Wk = np.asarray(Wk, np.float32)
    perms_len = np.asarray(perms_len)
    perm_L = np.asarray(perm_L)
    perm_B = np.asarray(perm_B)

    nc = _build_program()
    in_maps = _make_inputs(c, z, Wk, perms_len, perm_L, perm_B)
    res = bass_utils.run_bass_kernel_spmd(
        nc, in_maps, core_ids=list(range(2 * B)), trace=_trace
    )
    if _result_holder is not None:
        _result_holder.append(res)

    sigma_g = np.argsort(perm_L)
    out = np.empty((B, NM, LW, K), np.float32)
    for b in range(B):
        for h in range(2):
            L0 = L0S[h]
            sc = res.results[2 * b + h]["outs"].reshape(NM, K, LH)
            if h == 0:
                ii = np.arange(0, 250)
            else:
                ii = np.arange(250 - L0, LH)
            l_loc = L0 + ii
            # pos + neg_len: identity l-order
            out[b, 0, l_loc, :] = sc[0, :, ii]
            for q in range(B):
                out[b, 1 + B + q, l_loc, :] = sc[5 + q, :, ii]
            # neg_g: j-order scatter
            l_of_j = sigma_g[l_loc]
            for n in range(B):
                out[b, 1 + n, l_of_j, :] = sc[1 + n, :, ii]
    return out


# revision 30
# speedup vs baseline: 2.3019x; 1.0841x over previous
"""Trainium2 Bass kernel for nn_PredictionModel (CPC-style prediction scores).

Computation (B=4, L=512, D=512, C=256, K=12, LW=500):
  c_proj[b,l,k,d] = sum_c Wk[k,d,c] * c[b,l,c]          (l < LW)
  zw[b,l,k,d]     = z[b, l+1+k, d]
  pos[b,l,k]      = <c_proj[b,l,k], zw[b,l,k]>
  neg_g[b,n,l,k]  = <c_proj[b,l,k], zw[perm_B[n], perm_L[l], k]>
  neg_len[b,n,l,k]= <c_proj[b,l,k], zw[b, perms_len[n,l], k]>
  out = concat([pos[:,None], neg_g, neg_len], axis=1)   # (B, 9, LW, K)

Sharding: 8 cores = 4 batches x 2 ranges ([0,256) and [244,500); host takes
position <250 from half 0 and >=250 from half 1).

All 9 score sets are computed in d-partition layout: products
P[d, k, l] = cprojT[d, k, l] * window[d, k, l] on DVE/Pool, then the
d-reduction runs on PE as indicator-column matmuls that accumulate every
(group, dchunk) product into ONE PSUM [9, K*LH] scores tile
(row = score set).

Window sources:
 * pos: in-place overlapping AP on resident zT[b] (k,l strides both 1).
 * neg_g: in-place overlapping AP on resident zT[perm_B[n]], evaluated in
   source-row order j with cprojG from host-permuted c (sigma_g =
   argsort(perm_L)); host scatters j->l afterwards (free).
 * neg_len: host-gathered d-part window tiles, DMA-streamed.

cprojT versions (identity + sigma_g) are computed by PE matmuls straight
into [d, k, l] layout chunks and cast to bf16 via ACT copies.
"""

import numpy as np
import ml_dtypes

import bass_rust
import concourse.mybir as mybir
from concourse import bacc
from concourse.tile import TileContext
from concourse import bass_utils

B, L, D, C, K = 4, 512, 512, 256, 12
LW = L - K          # 500
LH = 256            # padded per-core l (and j) count
L0S = [0, 244]      # absolute start of each half
NM = 2 * B + 1      # 9 score rows per (l, k)
F32 = mybir.dt.float32
BF16 = mybir.dt.bfloat16
BF16_NP = ml_dtypes.bfloat16

NDC = D // 128      # 4 d-part chunks
FREE = K * LH       # 3072 flattened (k, l) columns

_NC = None

# score-row order: 0=pos, 1..4=neg_g (j-order), 5..8=neg_len
# pool_mults: set of (row, dc) chunk-mult assignments run on Pool instead
# of DVE.
CFG = {
    # (row, dc, third) mults run on Pool when (row in pool_rows and
    # third in pool_thirds); everything else on DVE. padd units pre-add
    # dc-pairs of products on DVE, halving their chain matmuls.
    "pool_rows": (2, 6),
    "pool_thirds": (0, 1, 2),
    "padd_rows": (1, 3),
    "padd_thirds": (0, 1, 2),
    "zw_bufs": 8,
    "prodg_bufs": 12,
    "copy_rot": "a",      # version psum->sbuf copy engine rotation
}


def _win(zt_sb, col, base, nk, nj):
    """Overlapping-window AP [128, nk, nj] over zt_sb[:, col, base:]:
    element (d, k, j) -> zt_sb[d, col, base + k + j]."""
    ap = zt_sb[:, col, base:].copy()
    part = ap.ap[0]
    ap.ap = bass_rust.VecI64Pair([list(part), [1, nk], [1, nj]])
    return ap


def _build_program(cfg=None):
    """One NeuronCore program, identical across the 8 cores."""
    global _NC
    if cfg is None and _NC is not None:
        return _NC
    cfg = {**CFG, **(cfg or {})}
    nc = bacc.Bacc()
    ct_d = nc.dram_tensor("ct", [128, 2, LH], BF16, kind="ExternalInput")
    ctg_d = nc.dram_tensor("ctg", [128, 2, LH], BF16, kind="ExternalInput")
    wkt_d = nc.dram_tensor("wkt", [128, K, 2, D], BF16, kind="ExternalInput")
    # z transposed: slots 0..3 = z[perm_B[n]], slot 4 = z[b] (own batch)
    zt_d = nc.dram_tensor("zt", [NDC, B + 1, 128, L], BF16, kind="ExternalInput")
    # host-gathered d-part windows for neg_len: [q 4, dc 4, d 128, k, l]
    zw_d = nc.dram_tensor("zw", [B, NDC, 128, K, LH], BF16, kind="ExternalInput")
    # all scores: row 0=pos, 1..4=neg_g, 5..8=neg_len; columns (k, l|j)
    outs_d = nc.dram_tensor("outs", [NM, FREE], F32, kind="ExternalOutput")

    pool_rows = set(cfg["pool_rows"])
    pool_thirds = set(cfg["pool_thirds"])
    NT = 3                    # k-thirds
    KT = K // NT              # 4 k's per third
    TFREE = KT * LH           # 1024 columns per third

    with TileContext(nc) as tc:
        with (
            tc.tile_pool(name="const", bufs=1) as const_pool,
            tc.tile_pool(name="psum", bufs=cfg.get("psv_bufs", 6), space="PSUM")
            as psum_pool,
            tc.tile_pool(name="psq", bufs=2, space="PSUM") as psq_pool,
            tc.tile_pool(name="cpg", bufs=1) as cpg_pool,
            tc.tile_pool(name="zw", bufs=cfg["zw_bufs"]) as zw_pool,
            tc.tile_pool(name="prodg", bufs=cfg["prodg_bufs"]) as prodg_pool,
            tc.tile_pool(name="sgc", bufs=3) as sgc_pool,
        ):
            ct_sb = const_pool.tile([128, 2, LH], BF16, tag="ct", name="ct_sb")
            ctg_sb = const_pool.tile([128, 2, LH], BF16, tag="ctg", name="ctg_sb")
            wkt_sb = const_pool.tile([128, K, 2, D], BF16, tag="wkt", name="wkt_sb")
            # first version kpairs need wkt k 0-3 + ct/ctg only
            nc.sync.dma_start(out=wkt_sb[:, 0:4], in_=wkt_d[:, 0:4])
            nc.sync.dma_start(out=ct_sb[:], in_=ct_d[:])
            nc.sync.dma_start(out=ctg_sb[:], in_=ctg_d[:])
            for kc in range(1, 3):
                nc.sync.dma_start(
                    out=wkt_sb[:, kc * 4 : (kc + 1) * 4],
                    in_=wkt_d[:, kc * 4 : (kc + 1) * 4],
                )
            zt_sb = const_pool.tile(
                [128, NDC * (B + 1), L], BF16, tag="zt", name="zt_sb"
            )

            def load_zt(dc):
                nc.sync.dma_start(
                    out=zt_sb[:, dc * (B + 1) : (dc + 1) * (B + 1)],
                    in_=zt_d[dc].rearrange("s p r -> p s r"),
                )
            # indicator columns: e9s[r][:, r] = 1
            e9s = []
            for r in range(NM):
                t = const_pool.tile([128, NM], BF16, tag=f"e9_{r}", name=f"e9_{r}")
                nc.vector.memset(t[:], 0)
                nc.vector.memset(t[:, r : r + 1], 1.0)
                e9s.append(t)

            # ---- cprojT versions: [d 128, k, l] bf16 per dchunk, built
            # just-in-time per k-third inside the main loop ----
            copy_engs = {
                "a": lambda o, i: nc.scalar.copy(o, i),
                "d": lambda o, i: nc.vector.tensor_copy(out=o, in_=i),
                "p": lambda o, i: nc.gpsimd.tensor_copy(out=o, in_=i),
            }
            rot = cfg["copy_rot"]
            cp_n = [0]

            cpi = [
                cpg_pool.tile([128, K, LH], BF16, tag=f"cpi{dc}", name=f"cpi{dc}")
                for dc in range(NDC)
            ]
            cpg = [
                cpg_pool.tile([128, K, LH], BF16, tag=f"cpg{dc}", name=f"cpg{dc}")
                for dc in range(NDC)
            ]

            def build_third(tiles, src_sb, dc, t):
                """Version matmuls + copy for k in [t*KT, (t+1)*KT) of chunk dc."""
                tile = tiles[dc]
                for kp in range(t * KT // 2, (t + 1) * KT // 2):
                    psv = psum_pool.tile(
                        [128, 2, LH], F32, name=f"psv{id(tiles)}_{dc}_{kp}",
                        tag="ps",
                    )
                    for k2 in range(2):
                        for ci in range(2):
                            nc.tensor.matmul(
                                psv[:, k2],
                                wkt_sb[:, kp * 2 + k2, ci,
                                       dc * 128 : (dc + 1) * 128],
                                src_sb[:, ci, :],
                                start=(ci == 0),
                                stop=(ci == 1),
                            )
                    eng = copy_engs[rot[cp_n[0] % len(rot)]]
                    eng(tile[:, kp * 2 : (kp + 1) * 2, :], psv[:])
                    cp_n[0] += 1

            # ---- neg_len window streams, loaded per (t, dc, q) ----
            zw_tiles = {}

            def load_zw(q, dc, t):
                tl = zw_pool.tile(
                    [128, KT, LH], BF16, tag="zw", name=f"zw{q}_{dc}_{t}"
                )
                nc.sync.dma_start(
                    out=tl[:], in_=zw_d[q, dc, :, t * KT : (t + 1) * KT, :]
                )
                zw_tiles[(q, dc, t)] = tl

            # ---- products + indicator-matmul reduction, by k-thirds ----
            # a matmul's output must fit one PSUM bank (512 f32), so each
            # third accumulates into two k-sixth tiles [9, 512] which are
            # drained at the third boundary and recycled. For padd units,
            # dc-pairs of products are pre-added on DVE/Pool so the chain
            # needs half the matmuls.
            padd_rows = set(cfg.get("padd_rows", ()))
            padd_thirds = set(cfg.get("padd_thirds", ()))
            padd_eng = cfg.get("padd_eng", "d")

            def is_padd(row, t):
                return row in padd_rows and t in padd_thirds

            psqs = {}
            nmm = {}
            nchain = {}
            stash = {}

            def chain_mm(row, t, tile, ap3):
                for h in range(2):
                    s = 2 * t + h
                    nc.tensor.matmul(
                        psqs[s][:],
                        e9s[row][:],
                        ap3[:, h * 2 : (h + 1) * 2, :].rearrange(
                            "p k j -> p (k j)"
                        ),
                        start=(nmm[s] == 0),
                        stop=(nmm[s] == nchain[s] - 1),
                    )
                    nmm[s] += 1

            def unit(row, dc, t, cp, win_t):
                """One (row, dchunk, k-third): mult (+ pair-add) + chain."""
                prodg = prodg_pool.tile(
                    [128, KT, LH], BF16, tag="prodg", name=f"pg{row}_{dc}_{t}"
                )
                on_pool = row in pool_rows and t in pool_thirds
                eng = nc.gpsimd if on_pool else nc.vector
                eng.tensor_tensor(
                    out=prodg[:],
                    in0=cp[dc][:, t * KT : (t + 1) * KT, :],
                    in1=win_t,
                    op=mybir.AluOpType.mult,
                )
                if not is_padd(row, t):
                    chain_mm(row, t, prodg, prodg)
                    return
                if dc % 2 == 0:
                    stash[(row, t)] = prodg
                    return
                prev = stash.pop((row, t))
                ssum = prodg_pool.tile(
                    [128, KT, LH], BF16, tag="prodg", name=f"ps{row}_{dc}_{t}"
                )
                aeng = nc.gpsimd if padd_eng == "p" else nc.vector
                aeng.tensor_tensor(
                    out=ssum[:], in0=prev[:], in1=prodg[:],
                    op=mybir.AluOpType.add,
                )
                chain_mm(row, t, ssum, ssum)

            # zt slot column index for (slot, dc) in the dc-major zt_sb
            def zslot(slot, dc):
                return dc * (B + 1) + slot

            # DMA order: zt chunks and zw slices interleaved in consumption
            # order (t-outer, dc-inner)
            for dc in range(NDC):
                load_zt(dc)
                for q in range(B):
                    load_zw(q, dc, 0)
            for t in range(1, NT):
                for dc in range(NDC):
                    for q in range(B):
                        load_zw(q, dc, t)

            # main loop: t outer, dc inner; windows per (row, dc, third):
            # in-place APs start at column 1 + t*KT (k offset folds into the
            # window base); streamed tiles are per-third.
            for t in range(NT):
                for h in range(2):
                    s = 2 * t + h
                    psqs[s] = psq_pool.tile(
                        [NM, 2 * LH], F32, name=f"psq{s}", tag="psq"
                    )
                    nmm[s] = 0
                    nchain[s] = sum(
                        NDC // 2 if is_padd(row, t) else NDC
                        for row in range(NM)
                    )
                for dc in range(NDC):
                    build_third(cpi, ct_sb, dc, t)
                    build_third(cpg, ctg_sb, dc, t)
                    rows = [
                        (0, cpi, lambda dc=dc: _win(
                            zt_sb, zslot(B, dc), 1 + t * KT, KT, LH))
                    ] + [
                        (1 + n, cpg, lambda dc=dc, n=n: _win(
                            zt_sb, zslot(n, dc), 1 + t * KT, KT, LH))
                        for n in range(B)
                    ] + [
                        (5 + q, cpi, lambda dc=dc, q=q, t=t: zw_tiles[
                            (q, dc, t)][:])
                        for q in range(B)
                    ]
                    rows.sort(key=lambda r: (r[0] in pool_rows
                                             and t in pool_thirds))
                    for row, cp, winf in rows:
                        unit(row, dc, t, cp, winf())
                # drain this third's two sixths: PSUM -> SBUF -> DRAM
                for h in range(2):
                    s = 2 * t + h
                    sgc = sgc_pool.tile(
                        [NM, 2 * LH], F32, tag="sgc", name=f"sgc{s}"
                    )
                    nc.scalar.copy(sgc[:], psqs[s][:])
                    nc.sync.dma_start(
                        out=outs_d.rearrange("m (k j) -> m k j", k=K)[
                            :, s * 2 : (s + 1) * 2, :
                        ],
                        in_=sgc[:],
                    )

    nc.compile()
    if cfg == CFG:
        _NC = nc
    return nc


def _make_inputs(c, z, Wk, perms_len, perm_L, perm_B):
    """Host-side sharding: per-core input dicts."""
    z_bf = z.astype(BF16_NP)
    wkt = np.ascontiguousarray(
        Wk.transpose(0, 2, 1).reshape(K, 2, 128, D).transpose(2, 0, 1, 3)
    ).astype(BF16_NP)  # [128, K, 2, D]
    # zT chunks: [NDC, 128, L] per batch
    ztc = np.ascontiguousarray(z_bf.transpose(0, 2, 1).reshape(B, NDC, 128, L))
    perm_B = np.asarray(perm_B, np.int64)
    sigma_g = np.argsort(perm_L).astype(np.int64)  # l = sigma_g[j]
    karr = np.arange(K, dtype=np.int64)
    in_maps = []
    for b in range(B):
        for h in range(2):
            L0 = L0S[h]
            l_abs = np.arange(L0, L0 + LH, dtype=np.int64)

            def ctr(cols):
                return np.ascontiguousarray(
                    c[b, cols, :].T.reshape(2, 128, LH).transpose(1, 0, 2)
                ).astype(BF16_NP)

            ct = ctr(l_abs)
            ctg = ctr(sigma_g[l_abs])
            # zt slots 0..3 = z[perm_B[n]], slot 4 = z[b]; all shifted by L0
            # so window column (1 + j + k) reads z[sb, L0 + j + 1 + k].
            # Layout dc-major: [NDC, B+1, 128, L]
            zt = np.empty((NDC, B + 1, 128, L), dtype=BF16_NP)
            for slot in range(B + 1):
                sb = int(perm_B[slot]) if slot < B else b
                sh = np.zeros((L, 512), dtype=BF16_NP)
                sh[: L - L0] = z_bf[sb, L0:]
                zt[:, slot] = sh.T.reshape(NDC, 128, L)
            # neg_len windows, d-part [q, dc, d, k, l]
            zw = np.empty((B, NDC, 128, K, LH), dtype=BF16_NP)
            for q in range(B):
                sl = perms_len[q, l_abs].astype(np.int64)  # (LH,)
                rows = sl[None, :] + 1 + karr[:, None]     # (K, LH)
                g = z_bf[b, rows]                          # (K, LH, 512)
                zw[q] = g.transpose(2, 0, 1).reshape(NDC, 128, K, LH)
            in_maps.append(
                {"ct": ct, "ctg": ctg, "wkt": wkt, "zw": zw, "zt": zt}
            )
    return in_maps


def kernel(c, z, Wk, perms_len, perm_L, perm_B, _trace=False, _result_holder=None):
    c = np.asarray(c, np.float32)
    z = np.asarray(z, np.float32)
    Wk = np.asarray(Wk, np.float32)
    perms_len = np.asarray(perms_len)
    perm_L = np.asarray(perm_L)
    perm_B = np.asarray(perm_B)

    nc = _build_program()
    in_maps = _make_inputs(c, z, Wk, perms_len, perm_L, perm_B)
    res = bass_utils.run_bass_kernel_spmd(
        nc, in_maps, core_ids=list(range(2 * B)), trace=_trace
    )
    if _result_holder is not None:
        _result_holder.append(res)

    sigma_g = np.argsort(perm_L)
    out = np.empty((B, NM, LW, K), np.float32)
    for b in range(B):
        for h in range(2):
            L0 = L0S[h]
            sc = res.results[2 * b + h]["outs"].reshape(NM, K, LH)
            if h == 0:
                ii = np.arange(0, 250)
            else:
                ii = np.arange(250 - L0, LH)
            l_loc = L0 + ii
            # pos + neg_len: identity l-order
            out[b, 0, l_loc, :] = sc[0, :, ii]
            for q in range(B):
                out[b, 1 + B + q, l_loc, :] = sc[5 + q, :, ii]
            # neg_g: j-order scatter
            l_of_j = sigma_g[l_loc]
            for n in range(B):
                out[b, 1 + n, l_of_j, :] = sc[1 + n, :, ii]
    return out


# revision 36
# speedup vs baseline: 2.4258x; 1.0538x over previous
"""Trainium2 Bass kernel for nn_PredictionModel (CPC-style prediction scores).

Computation (B=4, L=512, D=512, C=256, K=12, LW=500):
  c_proj[b,l,k,d] = sum_c Wk[k,d,c] * c[b,l,c]          (l < LW)
  zw[b,l,k,d]     = z[b, l+1+k, d]
  pos[b,l,k]      = <c_proj[b,l,k], zw[b,l,k]>
  neg_g[b,n,l,k]  = <c_proj[b,l,k], zw[perm_B[n], perm_L[l], k]>
  neg_len[b,n,l,k]= <c_proj[b,l,k], zw[b, perms_len[n,l], k]>
  out = concat([pos[:,None], neg_g, neg_len], axis=1)   # (B, 9, LW, K)

Sharding: 8 cores = 4 batches x 2 ranges ([0,256) and [244,500); host takes
position <250 from half 0 and >=250 from half 1).

All 9 score sets are computed in d-partition layout: products
P[d, k, l] = cprojT[d, k, l] * window[d, k, l] on DVE/Pool, then the
d-reduction runs on PE as indicator-column matmuls that accumulate every
(group, dchunk) product into ONE PSUM [9, K*LH] scores tile
(row = score set).

Window sources:
 * pos: in-place overlapping AP on resident zT[b] (k,l strides both 1).
 * neg_g: in-place overlapping AP on resident zT[perm_B[n]], evaluated in
   source-row order j with cprojG from host-permuted c (sigma_g =
   argsort(perm_L)); host scatters j->l afterwards (free).
 * neg_len: host-gathered d-part window tiles, DMA-streamed.

cprojT versions (identity + sigma_g) are computed by PE matmuls straight
into [d, k, l] layout chunks and cast to bf16 via ACT copies.
"""

import numpy as np
import ml_dtypes

import bass_rust
import concourse.mybir as mybir
from concourse import bacc
from concourse.tile import TileContext
from concourse import bass_utils

B, L, D, C, K = 4, 512, 512, 256, 12
LW = L - K          # 500
LH = 256            # padded per-core l (and j) count
L0S = [0, 244]      # absolute start of each half
NM = 2 * B + 1      # 9 score rows per (l, k)
F32 = mybir.dt.float32
BF16 = mybir.dt.bfloat16
BF16_NP = ml_dtypes.bfloat16

NDC = D // 128      # 4 d-part chunks
FREE = K * LH       # 3072 flattened (k, l) columns

_NC = None

# score-row order: 0=pos, 1..4=neg_g (j-order), 5..8=neg_len
# pool_mults: set of (row, dc) chunk-mult assignments run on Pool instead
# of DVE.
CFG = {
    # (row, dc, third) mults run on Pool when (row in pool_rows and
    # third in pool_thirds); everything else on DVE. padd units pre-add
    # dc-pairs of products on DVE, halving their chain matmuls.
    "pool_rows": (2, 6),
    "pool_thirds": (0, 1, 2),
    "padd_rows": (1, 3),
    "padd_thirds": (0, 1, 2),
    "zw_bufs": 12,
    "prodg_bufs": 20,
    "copy_rot": "a",      # version psum->sbuf copy engine rotation
}


def _win(zt_sb, col, base, nk, nj):
    """Overlapping-window AP [128, nk, nj] over zt_sb[:, col, base:]:
    element (d, k, j) -> zt_sb[d, col, base + k + j]."""
    ap = zt_sb[:, col, base:].copy()
    part = ap.ap[0]
    ap.ap = bass_rust.VecI64Pair([list(part), [1, nk], [1, nj]])
    return ap


def _build_program(cfg=None):
    """One NeuronCore program, identical across the 8 cores."""
    global _NC
    if cfg is None and _NC is not None:
        return _NC
    cfg = {**CFG, **(cfg or {})}
    nc = bacc.Bacc()
    ct_d = nc.dram_tensor("ct", [128, 2, LH], BF16, kind="ExternalInput")
    ctg_d = nc.dram_tensor("ctg", [128, 2, LH], BF16, kind="ExternalInput")
    wkt_d = nc.dram_tensor("wkt", [128, K, 2, D], BF16, kind="ExternalInput")
    # z transposed: slots 0..3 = z[perm_B[n]], slot 4 = z[b] (own batch)
    zt_d = nc.dram_tensor("zt", [NDC, B + 1, 128, L], BF16, kind="ExternalInput")
    # host-gathered d-part windows for neg_len: [q 4, dc 4, d 128, k, l]
    zw_d = nc.dram_tensor("zw", [B, NDC, 128, K, LH], BF16, kind="ExternalInput")
    # all scores: row 0=pos, 1..4=neg_g, 5..8=neg_len; columns (k, l|j)
    outs_d = nc.dram_tensor("outs", [NM, FREE], F32, kind="ExternalOutput")

    pool_rows = set(cfg["pool_rows"])
    pool_thirds = set(cfg["pool_thirds"])
    NT = 3                    # k-thirds
    KT = K // NT              # 4 k's per third
    TFREE = KT * LH           # 1024 columns per third

    with TileContext(nc) as tc:
        with (
            tc.tile_pool(name="const", bufs=1) as const_pool,
            tc.tile_pool(name="psum", bufs=cfg.get("psv_bufs", 6), space="PSUM")
            as psum_pool,
            tc.tile_pool(name="psq", bufs=2, space="PSUM") as psq_pool,
            tc.tile_pool(name="cpg", bufs=1) as cpg_pool,
            tc.tile_pool(name="zw", bufs=cfg["zw_bufs"]) as zw_pool,
            tc.tile_pool(name="prodg", bufs=cfg["prodg_bufs"]) as prodg_pool,
            tc.tile_pool(name="sgc", bufs=3) as sgc_pool,
        ):
            ct_sb = const_pool.tile([128, 2, LH], BF16, tag="ct", name="ct_sb")
            ctg_sb = const_pool.tile([128, 2, LH], BF16, tag="ctg", name="ctg_sb")
            wkt_sb = const_pool.tile([128, K, 2, D], BF16, tag="wkt", name="wkt_sb")
            # first version kpairs need wkt k 0-3 + ct/ctg only
            nc.sync.dma_start(out=wkt_sb[:, 0:2], in_=wkt_d[:, 0:2])
            nc.sync.dma_start(out=ctg_sb[:], in_=ctg_d[:])
            nc.sync.dma_start(out=ct_sb[:], in_=ct_d[:])
            nc.sync.dma_start(out=wkt_sb[:, 2:4], in_=wkt_d[:, 2:4])

            zt_sb = const_pool.tile(
                [128, NDC * (B + 1), L], BF16, tag="zt", name="zt_sb"
            )

            def load_zt(dc):
                nc.sync.dma_start(
                    out=zt_sb[:, dc * (B + 1) : (dc + 1) * (B + 1)],
                    in_=zt_d[dc].rearrange("s p r -> p s r"),
                )
            # indicator columns: e9s[r][:, r] = 1
            e9s = []
            for r in range(NM):
                t = const_pool.tile([128, NM], BF16, tag=f"e9_{r}", name=f"e9_{r}")
                nc.vector.memset(t[:], 0)
                nc.vector.memset(t[:, r : r + 1], 1.0)
                e9s.append(t)

            # ---- cprojT versions: [d 128, k, l] bf16 per dchunk, built
            # just-in-time per k-third inside the main loop ----
            copy_engs = {
                "a": lambda o, i: nc.scalar.copy(o, i),
                "d": lambda o, i: nc.vector.tensor_copy(out=o, in_=i),
                "p": lambda o, i: nc.gpsimd.tensor_copy(out=o, in_=i),
            }
            rot = cfg["copy_rot"]
            cp_n = [0]

            cpi = [
                cpg_pool.tile([128, K, LH], BF16, tag=f"cpi{dc}", name=f"cpi{dc}")
                for dc in range(NDC)
            ]
            cpg = [
                cpg_pool.tile([128, K, LH], BF16, tag=f"cpg{dc}", name=f"cpg{dc}")
                for dc in range(NDC)
            ]

            def build_third(tiles, src_sb, dc, t):
                """Version matmuls + copy for k in [t*KT, (t+1)*KT) of chunk dc."""
                tile = tiles[dc]
                for kp in range(t * KT // 2, (t + 1) * KT // 2):
                    psv = psum_pool.tile(
                        [128, 2, LH], F32, name=f"psv{id(tiles)}_{dc}_{kp}",
                        tag="ps",
                    )
                    for k2 in range(2):
                        for ci in range(2):
                            nc.tensor.matmul(
                                psv[:, k2],
                                wkt_sb[:, kp * 2 + k2, ci,
                                       dc * 128 : (dc + 1) * 128],
                                src_sb[:, ci, :],
                                start=(ci == 0),
                                stop=(ci == 1),
                            )
                    eng = copy_engs[rot[cp_n[0] % len(rot)]]
                    eng(tile[:, kp * 2 : (kp + 1) * 2, :], psv[:])
                    cp_n[0] += 1

            # ---- neg_len window streams, loaded per (t, dc, q) ----
            zw_tiles = {}

            def load_zw(q, dc, t):
                tl = zw_pool.tile(
                    [128, KT, LH], BF16, tag="zw", name=f"zw{q}_{dc}_{t}"
                )
                nc.sync.dma_start(
                    out=tl[:], in_=zw_d[q, dc, :, t * KT : (t + 1) * KT, :]
                )
                zw_tiles[(q, dc, t)] = tl

            # ---- products + indicator-matmul reduction, by k-thirds ----
            # a matmul's output must fit one PSUM bank (512 f32), so each
            # third accumulates into two k-sixth tiles [9, 512] which are
            # drained at the third boundary and recycled. For padd units,
            # dc-pairs of products are pre-added on DVE/Pool so the chain
            # needs half the matmuls.
            padd_rows = set(cfg.get("padd_rows", ()))
            padd_thirds = set(cfg.get("padd_thirds", ()))
            padd_eng = cfg.get("padd_eng", "d")

            def is_padd(row, t):
                return row in padd_rows and t in padd_thirds

            psqs = {}
            nmm = {}
            nchain = {}
            stash = {}

            def chain_mm(row, t, tile, ap3):
                for h in range(2):
                    s = 2 * t + h
                    nc.tensor.matmul(
                        psqs[s][:],
                        e9s[row][:],
                        ap3[:, h * 2 : (h + 1) * 2, :].rearrange(
                            "p k j -> p (k j)"
                        ),
                        start=(nmm[s] == 0),
                        stop=(nmm[s] == nchain[s] - 1),
                    )
                    nmm[s] += 1

            def unit(row, dc, t, cp, win_t):
                """One (row, dchunk, k-third): mult (+ pair-add) + chain."""
                prodg = prodg_pool.tile(
                    [128, KT, LH], BF16, tag="prodg", name=f"pg{row}_{dc}_{t}"
                )
                on_pool = (row in pool_rows and t in pool_thirds
                           and not (t == NT - 1 and dc == NDC - 1))
                eng = nc.gpsimd if on_pool else nc.vector
                eng.tensor_tensor(
                    out=prodg[:],
                    in0=cp[dc][:, t * KT : (t + 1) * KT, :],
                    in1=win_t,
                    op=mybir.AluOpType.mult,
                )
                if not is_padd(row, t):
                    chain_mm(row, t, prodg, prodg)
                    return
                if dc % 2 == 0:
                    stash[(row, t)] = prodg
                    return
                prev = stash.pop((row, t))
                ssum = prodg_pool.tile(
                    [128, KT, LH], BF16, tag="prodg", name=f"ps{row}_{dc}_{t}"
                )
                aeng = nc.gpsimd if padd_eng == "p" else nc.vector
                aeng.tensor_tensor(
                    out=ssum[:], in0=prev[:], in1=prodg[:],
                    op=mybir.AluOpType.add,
                )
                chain_mm(row, t, ssum, ssum)

            # zt slot column index for (slot, dc) in the dc-major zt_sb
            def zslot(slot, dc):
                return dc * (B + 1) + slot

            # DMA order: zt chunks and zw slices interleaved in consumption
            # order (t-outer, dc-inner); remaining wkt chunks after the
            # t0/dc0-dc1 data so the first phases start early
            for dc in range(NDC):
                load_zt(dc)
                for q in range(B):
                    load_zw(q, dc, 0)
                if dc >= 2:
                    nc.sync.dma_start(
                        out=wkt_sb[:, dc * 4 - 4 : dc * 4],
                        in_=wkt_d[:, dc * 4 - 4 : dc * 4],
                    )
            for t in range(1, NT):
                for dc in range(NDC):
                    for q in range(B):
                        load_zw(q, dc, t)

            # main loop: t outer, dc inner; windows per (row, dc, third):
            # in-place APs start at column 1 + t*KT (k offset folds into the
            # window base); streamed tiles are per-third.
            for t in range(NT):
                for h in range(2):
                    s = 2 * t + h
                    psqs[s] = psq_pool.tile(
                        [NM, 2 * LH], F32, name=f"psq{s}", tag="psq"
                    )
                    nmm[s] = 0
                    nchain[s] = sum(
                        NDC // 2 if is_padd(row, t) else NDC
                        for row in range(NM)
                    )
                for dc in range(NDC):
                    build_third(cpi, ct_sb, dc, t)
                    build_third(cpg, ctg_sb, dc, t)
                    rows = [
                        (0, cpi, lambda dc=dc: _win(
                            zt_sb, zslot(B, dc), 1 + t * KT, KT, LH))
                    ] + [
                        (1 + n, cpg, lambda dc=dc, n=n: _win(
                            zt_sb, zslot(n, dc), 1 + t * KT, KT, LH))
                        for n in range(B)
                    ] + [
                        (5 + q, cpi, lambda dc=dc, q=q, t=t: zw_tiles[
                            (q, dc, t)][:])
                        for q in range(B)
                    ]
                    rows.sort(key=lambda r: (r[0] in pool_rows
                                             and t in pool_thirds
                                             and not (t == NT - 1
                                                      and dc == NDC - 1)))
                    for row, cp, winf in rows:
                        unit(row, dc, t, cp, winf())
                # drain this third's two sixths: PSUM -> SBUF -> DRAM
                for h in range(2):
                    s = 2 * t + h
                    sgc = sgc_pool.tile(
                        [NM, 2 * LH], F32, tag="sgc", name=f"sgc{s}"
                    )
                    nc.scalar.copy(sgc[:], psqs[s][:])
                    nc.sync.dma_start(
                        out=outs_d.rearrange("m (k j) -> m k j", k=K)[
                            :, s * 2 : (s + 1) * 2, :
                        ],
                        in_=sgc[:],
                    )

    nc.compile()
    if cfg == CFG:
        _NC = nc
    return nc


def _make_inputs(c, z, Wk, perms_len, perm_L, perm_B):
    """Host-side sharding: per-core input dicts."""
    z_bf = z.astype(BF16_NP)
    wkt = np.ascontiguousarray(
        Wk.transpose(0, 2, 1).reshape(K, 2, 128, D).transpose(2, 0, 1, 3)
    ).astype(BF16_NP)  # [128, K, 2, D]
    # zT chunks: [NDC, 128, L] per batch
    ztc = np.ascontiguousarray(z_bf.transpose(0, 2, 1).reshape(B, NDC, 128, L))
    perm_B = np.asarray(perm_B, np.int64)
    sigma_g = np.argsort(perm_L).astype(np.int64)  # l = sigma_g[j]
    karr = np.arange(K, dtype=np.int64)
    in_maps = []
    for b in range(B):
        for h in range(2):
            L0 = L0S[h]
            l_abs = np.arange(L0, L0 + LH, dtype=np.int64)

            def ctr(cols):
                return np.ascontiguousarray(
                    c[b, cols, :].T.reshape(2, 128, LH).transpose(1, 0, 2)
                ).astype(BF16_NP)

            ct = ctr(l_abs)
            ctg = ctr(sigma_g[l_abs])
            # zt slots 0..3 = z[perm_B[n]], slot 4 = z[b]; all shifted by L0
            # so window column (1 + j + k) reads z[sb, L0 + j + 1 + k].
            # Layout dc-major: [NDC, B+1, 128, L]
            zt = np.empty((NDC, B + 1, 128, L), dtype=BF16_NP)
            for slot in range(B + 1):
                sb = int(perm_B[slot]) if slot < B else b
                sh = np.zeros((L, 512), dtype=BF16_NP)
                sh[: L - L0] = z_bf[sb, L0:]
                zt[:, slot] = sh.T.reshape(NDC, 128, L)
            # neg_len windows, d-part [q, dc, d, k, l]
            zw = np.empty((B, NDC, 128, K, LH), dtype=BF16_NP)
            for q in range(B):
                sl = perms_len[q, l_abs].astype(np.int64)  # (LH,)
                rows = sl[None, :] + 1 + karr[:, None]     # (K, LH)
                g = z_bf[b, rows]                          # (K, LH, 512)
                zw[q] = g.transpose(2, 0, 1).reshape(NDC, 128, K, LH)
            in_maps.append(
                {"ct": ct, "ctg": ctg, "wkt": wkt, "zw": zw, "zt": zt}
            )
    return in_maps


def kernel(c, z, Wk, perms_len, perm_L, perm_B, _trace=False, _result_holder=None):
    c = np.asarray(c, np.float32)
    z = np.asarray(z, np.float32)
    Wk = np.asarray(Wk, np.float32)
    perms_len = np.asarray(perms_len)
    perm_L = np.asarray(perm_L)
    perm_B = np.asarray(perm_B)

    nc = _build_program()
    in_maps = _make_inputs(c, z, Wk, perms_len, perm_L, perm_B)
    res = bass_utils.run_bass_kernel_spmd(
        nc, in_maps, core_ids=list(range(2 * B)), trace=_trace
    )
    if _result_holder is not None:
        _result_holder.append(res)

    sigma_g = np.argsort(perm_L)
    out = np.empty((B, NM, LW, K), np.float32)
    for b in range(B):
        for h in range(2):
            L0 = L0S[h]
            sc = res.results[2 * b + h]["outs"].reshape(NM, K, LH)
            if h == 0:
                ii = np.arange(0, 250)
            else:
                ii = np.arange(250 - L0, LH)
            l_loc = L0 + ii
            # pos + neg_len: identity l-order
            out[b, 0, l_loc, :] = sc[0, :, ii]
            for q in range(B):
                out[b, 1 + B + q, l_loc, :] = sc[5 + q, :, ii]
            # neg_g: j-order scatter
            l_of_j = sigma_g[l_loc]
            for n in range(B):
                out[b, 1 + n, l_of_j, :] = sc[1 + n, :, ii]
    return out


# revision 37
# speedup vs baseline: 2.4647x; 1.0160x over previous
"""Trainium2 Bass kernel for nn_PredictionModel (CPC-style prediction scores).

Computation (B=4, L=512, D=512, C=256, K=12, LW=500):
  c_proj[b,l,k,d] = sum_c Wk[k,d,c] * c[b,l,c]          (l < LW)
  zw[b,l,k,d]     = z[b, l+1+k, d]
  pos[b,l,k]      = <c_proj[b,l,k], zw[b,l,k]>
  neg_g[b,n,l,k]  = <c_proj[b,l,k], zw[perm_B[n], perm_L[l], k]>
  neg_len[b,n,l,k]= <c_proj[b,l,k], zw[b, perms_len[n,l], k]>
  out = concat([pos[:,None], neg_g, neg_len], axis=1)   # (B, 9, LW, K)

Sharding: 8 cores = 4 batches x 2 ranges ([0,256) and [244,500); host takes
position <250 from half 0 and >=250 from half 1).

All 9 score sets are computed in d-partition layout: products
P[d, k, l] = cprojT[d, k, l] * window[d, k, l] on DVE/Pool, then the
d-reduction runs on PE as indicator-column matmuls that accumulate every
(group, dchunk) product into ONE PSUM [9, K*LH] scores tile
(row = score set).

Window sources:
 * pos: in-place overlapping AP on resident zT[b] (k,l strides both 1).
 * neg_g: in-place overlapping AP on resident zT[perm_B[n]], evaluated in
   source-row order j with cprojG from host-permuted c (sigma_g =
   argsort(perm_L)); host scatters j->l afterwards (free).
 * neg_len: host-gathered d-part window tiles, DMA-streamed.

cprojT versions (identity + sigma_g) are computed by PE matmuls straight
into [d, k, l] layout chunks and cast to bf16 via ACT copies.
"""

import numpy as np
import ml_dtypes

import bass_rust
import concourse.mybir as mybir
from concourse import bacc
from concourse.tile import TileContext
from concourse import bass_utils

B, L, D, C, K = 4, 512, 512, 256, 12
LW = L - K          # 500
LH = 250            # per-core l (and j) count
L0S = [0, 250]      # absolute start of each half
NM = 2 * B + 1      # 9 score rows per (l, k)
F32 = mybir.dt.float32
BF16 = mybir.dt.bfloat16
BF16_NP = ml_dtypes.bfloat16

NDC = D // 128      # 4 d-part chunks
FREE = K * LH       # 3072 flattened (k, l) columns

_NC = None

# score-row order: 0=pos, 1..4=neg_g (j-order), 5..8=neg_len
# pool_mults: set of (row, dc) chunk-mult assignments run on Pool instead
# of DVE.
CFG = {
    # (row, dc, third) mults run on Pool when (row in pool_rows and
    # third in pool_thirds); everything else on DVE. padd units pre-add
    # dc-pairs of products on DVE, halving their chain matmuls.
    "pool_rows": (2, 6),
    "pool_thirds": (0, 1, 2),
    "padd_rows": (1, 3),
    "padd_thirds": (0, 1, 2),
    "zw_bufs": 12,
    "prodg_bufs": 20,
    "copy_rot": "a",      # version psum->sbuf copy engine rotation
}


def _win(zt_sb, col, base, nk, nj):
    """Overlapping-window AP [128, nk, nj] over zt_sb[:, col, base:]:
    element (d, k, j) -> zt_sb[d, col, base + k + j]."""
    ap = zt_sb[:, col, base:].copy()
    part = ap.ap[0]
    ap.ap = bass_rust.VecI64Pair([list(part), [1, nk], [1, nj]])
    return ap


def _build_program(cfg=None):
    """One NeuronCore program, identical across the 8 cores."""
    global _NC
    if cfg is None and _NC is not None:
        return _NC
    cfg = {**CFG, **(cfg or {})}
    nc = bacc.Bacc()
    ct_d = nc.dram_tensor("ct", [128, 2, LH], BF16, kind="ExternalInput")
    ctg_d = nc.dram_tensor("ctg", [128, 2, LH], BF16, kind="ExternalInput")
    wkt_d = nc.dram_tensor("wkt", [128, K, 2, D], BF16, kind="ExternalInput")
    # z transposed: slots 0..3 = z[perm_B[n]], slot 4 = z[b] (own batch)
    zt_d = nc.dram_tensor("zt", [NDC, B + 1, 128, L], BF16, kind="ExternalInput")
    # host-gathered d-part windows for neg_len: [q 4, dc 4, d 128, k, l]
    zw_d = nc.dram_tensor("zw", [B, NDC, 128, K, LH], BF16, kind="ExternalInput")
    # all scores: row 0=pos, 1..4=neg_g, 5..8=neg_len; columns (k, l|j)
    outs_d = nc.dram_tensor("outs", [NM, FREE], F32, kind="ExternalOutput")

    pool_rows = set(cfg["pool_rows"])
    pool_thirds = set(cfg["pool_thirds"])
    NT = 3                    # k-thirds
    KT = K // NT              # 4 k's per third
    TFREE = KT * LH           # 1024 columns per third

    with TileContext(nc) as tc:
        with (
            tc.tile_pool(name="const", bufs=1) as const_pool,
            tc.tile_pool(name="psum", bufs=cfg.get("psv_bufs", 6), space="PSUM")
            as psum_pool,
            tc.tile_pool(name="psq", bufs=2, space="PSUM") as psq_pool,
            tc.tile_pool(name="cpg", bufs=1) as cpg_pool,
            tc.tile_pool(name="zw", bufs=cfg["zw_bufs"]) as zw_pool,
            tc.tile_pool(name="prodg", bufs=cfg["prodg_bufs"]) as prodg_pool,
            tc.tile_pool(name="sgc", bufs=3) as sgc_pool,
        ):
            ct_sb = const_pool.tile([128, 2, LH], BF16, tag="ct", name="ct_sb")
            ctg_sb = const_pool.tile([128, 2, LH], BF16, tag="ctg", name="ctg_sb")
            wkt_sb = const_pool.tile([128, K, 2, D], BF16, tag="wkt", name="wkt_sb")
            # first version kpairs need wkt k 0-3 + ct/ctg only
            nc.sync.dma_start(out=wkt_sb[:, 0:2], in_=wkt_d[:, 0:2])
            nc.sync.dma_start(out=ctg_sb[:], in_=ctg_d[:])
            nc.sync.dma_start(out=ct_sb[:], in_=ct_d[:])
            nc.sync.dma_start(out=wkt_sb[:, 2:4], in_=wkt_d[:, 2:4])

            zt_sb = const_pool.tile(
                [128, NDC * (B + 1), L], BF16, tag="zt", name="zt_sb"
            )

            def load_zt(dc):
                nc.sync.dma_start(
                    out=zt_sb[:, dc * (B + 1) : (dc + 1) * (B + 1)],
                    in_=zt_d[dc].rearrange("s p r -> p s r"),
                )
            # indicator columns: e9s[r][:, r] = 1
            e9s = []
            for r in range(NM):
                t = const_pool.tile([128, NM], BF16, tag=f"e9_{r}", name=f"e9_{r}")
                nc.vector.memset(t[:], 0)
                nc.vector.memset(t[:, r : r + 1], 1.0)
                e9s.append(t)

            # ---- cprojT versions: [d 128, k, l] bf16 per dchunk, built
            # just-in-time per k-third inside the main loop ----
            copy_engs = {
                "a": lambda o, i: nc.scalar.copy(o, i),
                "d": lambda o, i: nc.vector.tensor_copy(out=o, in_=i),
                "p": lambda o, i: nc.gpsimd.tensor_copy(out=o, in_=i),
            }
            rot = cfg["copy_rot"]
            cp_n = [0]

            cpi = [
                cpg_pool.tile([128, K, LH], BF16, tag=f"cpi{dc}", name=f"cpi{dc}")
                for dc in range(NDC)
            ]
            cpg = [
                cpg_pool.tile([128, K, LH], BF16, tag=f"cpg{dc}", name=f"cpg{dc}")
                for dc in range(NDC)
            ]

            def build_third(tiles, src_sb, dc, t):
                """Version matmuls + copy for k in [t*KT, (t+1)*KT) of chunk dc."""
                tile = tiles[dc]
                for kp in range(t * KT // 2, (t + 1) * KT // 2):
                    psv = psum_pool.tile(
                        [128, 2, LH], F32, name=f"psv{id(tiles)}_{dc}_{kp}",
                        tag="ps",
                    )
                    for k2 in range(2):
                        for ci in range(2):
                            nc.tensor.matmul(
                                psv[:, k2],
                                wkt_sb[:, kp * 2 + k2, ci,
                                       dc * 128 : (dc + 1) * 128],
                                src_sb[:, ci, :],
                                start=(ci == 0),
                                stop=(ci == 1),
                            )
                    eng = copy_engs[rot[cp_n[0] % len(rot)]]
                    eng(tile[:, kp * 2 : (kp + 1) * 2, :], psv[:])
                    cp_n[0] += 1

            # ---- neg_len window streams, loaded per (t, dc, q) ----
            zw_tiles = {}

            def load_zw(q, dc, t):
                tl = zw_pool.tile(
                    [128, KT, LH], BF16, tag="zw", name=f"zw{q}_{dc}_{t}"
                )
                nc.sync.dma_start(
                    out=tl[:], in_=zw_d[q, dc, :, t * KT : (t + 1) * KT, :]
                )
                zw_tiles[(q, dc, t)] = tl

            # ---- products + indicator-matmul reduction, by k-thirds ----
            # a matmul's output must fit one PSUM bank (512 f32), so each
            # third accumulates into two k-sixth tiles [9, 512] which are
            # drained at the third boundary and recycled. For padd units,
            # dc-pairs of products are pre-added on DVE/Pool so the chain
            # needs half the matmuls.
            padd_rows = set(cfg.get("padd_rows", ()))
            padd_thirds = set(cfg.get("padd_thirds", ()))
            padd_eng = cfg.get("padd_eng", "d")

            def is_padd(row, t):
                return row in padd_rows and t in padd_thirds

            psqs = {}
            nmm = {}
            nchain = {}
            stash = {}

            def chain_mm(row, t, tile, ap3):
                for h in range(2):
                    s = 2 * t + h
                    nc.tensor.matmul(
                        psqs[s][:],
                        e9s[row][:],
                        ap3[:, h * 2 : (h + 1) * 2, :].rearrange(
                            "p k j -> p (k j)"
                        ),
                        start=(nmm[s] == 0),
                        stop=(nmm[s] == nchain[s] - 1),
                    )
                    nmm[s] += 1

            def unit(row, dc, t, cp, win_t):
                """One (row, dchunk, k-third): mult (+ pair-add) + chain."""
                prodg = prodg_pool.tile(
                    [128, KT, LH], BF16, tag="prodg", name=f"pg{row}_{dc}_{t}"
                )
                on_pool = (row in pool_rows and t in pool_thirds
                           and not (t == NT - 1 and dc == NDC - 1))
                eng = nc.gpsimd if on_pool else nc.vector
                eng.tensor_tensor(
                    out=prodg[:],
                    in0=cp[dc][:, t * KT : (t + 1) * KT, :],
                    in1=win_t,
                    op=mybir.AluOpType.mult,
                )
                if not is_padd(row, t):
                    chain_mm(row, t, prodg, prodg)
                    return
                if dc % 2 == 0:
                    stash[(row, t)] = prodg
                    return
                prev = stash.pop((row, t))
                ssum = prodg_pool.tile(
                    [128, KT, LH], BF16, tag="prodg", name=f"ps{row}_{dc}_{t}"
                )
                aeng = nc.gpsimd if padd_eng == "p" else nc.vector
                aeng.tensor_tensor(
                    out=ssum[:], in0=prev[:], in1=prodg[:],
                    op=mybir.AluOpType.add,
                )
                chain_mm(row, t, ssum, ssum)

            # zt slot column index for (slot, dc) in the dc-major zt_sb
            def zslot(slot, dc):
                return dc * (B + 1) + slot

            # DMA order: zt chunks and zw slices interleaved in consumption
            # order (t-outer, dc-inner); remaining wkt chunks after the
            # t0/dc0-dc1 data so the first phases start early
            for dc in range(NDC):
                load_zt(dc)
                for q in range(B):
                    load_zw(q, dc, 0)
                if dc >= 2:
                    nc.sync.dma_start(
                        out=wkt_sb[:, dc * 4 - 4 : dc * 4],
                        in_=wkt_d[:, dc * 4 - 4 : dc * 4],
                    )
            for t in range(1, NT):
                for dc in range(NDC):
                    for q in range(B):
                        load_zw(q, dc, t)

            # main loop: t outer, dc inner; windows per (row, dc, third):
            # in-place APs start at column 1 + t*KT (k offset folds into the
            # window base); streamed tiles are per-third.
            for t in range(NT):
                for h in range(2):
                    s = 2 * t + h
                    psqs[s] = psq_pool.tile(
                        [NM, 2 * LH], F32, name=f"psq{s}", tag="psq"
                    )
                    nmm[s] = 0
                    nchain[s] = sum(
                        NDC // 2 if is_padd(row, t) else NDC
                        for row in range(NM)
                    )
                for dc in range(NDC):
                    build_third(cpi, ct_sb, dc, t)
                    build_third(cpg, ctg_sb, dc, t)
                    rows = [
                        (0, cpi, lambda dc=dc: _win(
                            zt_sb, zslot(B, dc), 1 + t * KT, KT, LH))
                    ] + [
                        (1 + n, cpg, lambda dc=dc, n=n: _win(
                            zt_sb, zslot(n, dc), 1 + t * KT, KT, LH))
                        for n in range(B)
                    ] + [
                        (5 + q, cpi, lambda dc=dc, q=q, t=t: zw_tiles[
                            (q, dc, t)][:])
                        for q in range(B)
                    ]
                    rows.sort(key=lambda r: (r[0] in pool_rows
                                             and t in pool_thirds
                                             and not (t == NT - 1
                                                      and dc == NDC - 1)))
                    for row, cp, winf in rows:
                        unit(row, dc, t, cp, winf())
                # drain this third's two sixths: PSUM -> SBUF -> DRAM
                for h in range(2):
                    s = 2 * t + h
                    sgc = sgc_pool.tile(
                        [NM, 2 * LH], F32, tag="sgc", name=f"sgc{s}"
                    )
                    nc.scalar.copy(sgc[:], psqs[s][:])
                    nc.sync.dma_start(
                        out=outs_d.rearrange("m (k j) -> m k j", k=K)[
                            :, s * 2 : (s + 1) * 2, :
                        ],
                        in_=sgc[:],
                    )

    nc.compile()
    if cfg == CFG:
        _NC = nc
    return nc


def _make_inputs(c, z, Wk, perms_len, perm_L, perm_B):
    """Host-side sharding: per-core input dicts."""
    z_bf = z.astype(BF16_NP)
    wkt = np.ascontiguousarray(
        Wk.transpose(0, 2, 1).reshape(K, 2, 128, D).transpose(2, 0, 1, 3)
    ).astype(BF16_NP)  # [128, K, 2, D]
    # zT chunks: [NDC, 128, L] per batch
    ztc = np.ascontiguousarray(z_bf.transpose(0, 2, 1).reshape(B, NDC, 128, L))
    perm_B = np.asarray(perm_B, np.int64)
    sigma_g = np.argsort(perm_L).astype(np.int64)  # l = sigma_g[j]
    karr = np.arange(K, dtype=np.int64)
    in_maps = []
    for b in range(B):
        for h in range(2):
            L0 = L0S[h]
            l_abs = np.arange(L0, L0 + LH, dtype=np.int64)

            def ctr(cols):
                return np.ascontiguousarray(
                    c[b, cols, :].T.reshape(2, 128, LH).transpose(1, 0, 2)
                ).astype(BF16_NP)

            ct = ctr(l_abs)
            ctg = ctr(sigma_g[l_abs])
            # zt slots 0..3 = z[perm_B[n]], slot 4 = z[b]; all shifted by L0
            # so window column (1 + j + k) reads z[sb, L0 + j + 1 + k].
            # Layout dc-major: [NDC, B+1, 128, L]
            zt = np.empty((NDC, B + 1, 128, L), dtype=BF16_NP)
            for slot in range(B + 1):
                sb = int(perm_B[slot]) if slot < B else b
                sh = np.zeros((L, 512), dtype=BF16_NP)
                sh[: L - L0] = z_bf[sb, L0:]
                zt[:, slot] = sh.T.reshape(NDC, 128, L)
            # neg_len windows, d-part [q, dc, d, k, l]
            zw = np.empty((B, NDC, 128, K, LH), dtype=BF16_NP)
            for q in range(B):
                sl = perms_len[q, l_abs].astype(np.int64)  # (LH,)
                rows = sl[None, :] + 1 + karr[:, None]     # (K, LH)
                g = z_bf[b, rows]                          # (K, LH, 512)
                zw[q] = g.transpose(2, 0, 1).reshape(NDC, 128, K, LH)
            in_maps.append(
                {"ct": ct, "ctg": ctg, "wkt": wkt, "zw": zw, "zt": zt}
            )
    return in_maps


def kernel(c, z, Wk, perms_len, perm_L, perm_B, _trace=False, _result_holder=None):
    c = np.asarray(c, np.float32)
    z = np.asarray(z, np.float32)
    Wk = np.asarray(Wk, np.float32)
    perms_len = np.asarray(perms_len)
    perm_L = np.asarray(perm_L)
    perm_B = np.asarray(perm_B)

    nc = _build_program()
    in_maps = _make_inputs(c, z, Wk, perms_len, perm_L, perm_B)
    res = bass_utils.run_bass_kernel_spmd(
        nc, in_maps, core_ids=list(range(2 * B)), trace=_trace
    )
    if _result_holder is not None:
        _result_holder.append(res)

    sigma_g = np.argsort(perm_L)
    out = np.empty((B, NM, LW, K), np.float32)
    for b in range(B):
        for h in range(2):
            L0 = L0S[h]
            sc = res.results[2 * b + h]["outs"].reshape(NM, K, LH)
            ii = np.arange(0, LH)
            l_loc = L0 + ii
            # pos + neg_len: identity l-order
            out[b, 0, l_loc, :] = sc[0, :, ii]
            for q in range(B):
                out[b, 1 + B + q, l_loc, :] = sc[5 + q, :, ii]
            # neg_g: j-order scatter
            l_of_j = sigma_g[l_loc]
            for n in range(B):
                out[b, 1 + n, l_of_j, :] = sc[1 + n, :, ii]
    return out
